# revision 25
# baseline (speedup 1.0000x reference)
"""MixProp GNN message-passing kernel for 8 TRN2 NeuronCores.

Reference computation (per batch element b):
    A_n = row_normalize(A + I)
    H_0 = X;  H_k = beta*X + (1-beta) * A_n @_nodes H_{k-1}   (k=1..3)
    out = W @_channels concat([H_0..H_3]) + bias

Kernel strategy (v3):
  - Data-parallel over batch: B=8 batch elements -> 8 cores, no collectives.
  - Host precomputes G_k s.t. H_k = G_k @ X, pre-casts operands to bf16 and
    pre-transposes X into the lhsT layout [wb, w, l, c]: the device does no
    layout work on X.
  - Per seq position l: per 128-node block, 3 column-packed matmuls (G1..G3)
    build PSUM H0[(hop,ch), v]; the column groups stream concurrently so
    each l costs ~4x512 moving columns (~850 ns warm).
  - The X part of the concat (H_0 = X itself, channels 0:32 of the conv)
    never touches the PE: the host supplies X^T pair-blocks and they are
    DMA'd straight into partitions 0:32 of the staging tile. (v3 computed
    X^T on the PE via identity matmuls; the tile scheduler sank those into
    the conv region where ~23 short instructions serialized at the ~50 ns
    dispatch floor, costing ~600 ns per pair.)
  - Seq positions are paired for the channel conv: both l's H0 go into one
    [128, 1024] SBUF tile (v-half-major), the conv is 2 matmuls of n=512
    into one PSUM bank, evacuated (+bias) in a single op.
  - PSUM->SBUF evacuation alternates DVE / Scalar engine per l.
  - Output staged bf16 as [(vh,o), (l, v)] per 32-l chunk, stored to a
    chunk-major DRAM layout (16 KB contiguous per partition); the host
    reassembles [64, 512, 256] f32 (bf16 output rounding ~0.4% rel, well
    inside the 2e-2 gate).
  - DMA order: first G tile lands in ~1 us and feeds ~48 HAM-warmup
    matmuls; X arrives in 64-l slices, first-needed first, so real compute
    starts ~12 us in instead of waiting for the full 10 MB fill.
"""

import sys

sys.path.insert(0, "/opt/trn_rl_repo")

import numpy as np

import concourse.bass as bass
import concourse.bacc as bacc
import concourse.mybir as mybir
from concourse import tile
from concourse import bass_utils

GDEP = 3
BETA = 0.05
C_IN = 32
C_OUT = 64
N = 512
B = 8
L = 256
NB = N // 128  # node blocks of 128

F32 = mybir.dt.float32
BF16 = mybir.dt.bfloat16


class CFG:
    def __init__(self, L=L, Lc=16, Lq=16, n_warm=48, xc_ahead=16):
        assert L % Lc == 0 and L % Lq == 0
        self.L = L
        self.Lc = Lc      # output store chunk
        self.Lq = Lq      # X load slice
        self.n_warm = n_warm
        self.xc_ahead = xc_ahead  # pairs of X^T DMA prefetch lookahead


def body(nc, tc, outs, ins, cfg: CFG):
    """Emit the per-core program. ins/outs are dicts of DRAM APs."""
    X_d = ins["xw"]         # [NB, 128, L, C_IN] bf16  pre-transposed lhsT
    XC_d = ins["xc"]        # [L//2, 2, 32, 512] bf16 X^T pair-blocks (vh)(l,v)
    G_d = ins["gt"]         # [GDEP, N, N] bf16  G_k^T
    W_d = ins["wt"]         # [128, C_OUT] bf16  W^T
    b_d = ins["bias2"]      # [128, 1]     f32   bias duplicated for (vh, o)
    out_d = outs["out"]     # [n_chunks, 128, Lc, 256] bf16 chunk-major

    Lc, Lq = cfg.Lc, cfg.Lq
    n_chunks = cfg.L // Lc
    n_xq = cfg.L // Lq

    XC_AHEAD = cfg.xc_ahead

    with (
        tc.tile_pool(name="const", bufs=1) as cpool,
        tc.tile_pool(name="h0sbA", bufs=XC_AHEAD + 2) as h0sbA_pool,
        tc.tile_pool(name="h0sbB", bufs=XC_AHEAD + 2) as h0sbB_pool,
        tc.tile_pool(name="outsb", bufs=2) as out_pool,
        tc.tile_pool(name="h0ps", bufs=3, space="PSUM") as h0ps_pool,
        tc.tile_pool(name="cvps", bufs=3, space="PSUM") as cvps_pool,
        tc.tile_pool(name="wmps", bufs=1, space="PSUM") as wm_pool,
    ):
        # ---- DMA order is load-bearing: g00 feeds the warmup matmuls ----
        g_t = [[None] * NB for _ in range(GDEP)]
        g_t[0][0] = cpool.tile([128, N], BF16, name="g0_0")
        nc.sync.dma_start(g_t[0][0][:], G_d[0, 0:128, :])

        w_t = cpool.tile([128, C_OUT], BF16, name="w_t")
        nc.sync.dma_start(w_t[:], W_d[:])
        b_t = cpool.tile([128, 1], F32, name="b_t")
        nc.sync.dma_start(b_t[:], b_d[:])

        # HAM warmup on the PE while the bulk DMAs land.
        wm = wm_pool.tile([128, N], F32, name="wm")
        for _ in range(cfg.n_warm):
            nc.tensor.matmul(
                wm[:], lhsT=g_t[0][0][:, 0:128], rhs=g_t[0][0][:],
                start=True, stop=True,
            )

        # X slices needed first, then the remaining G tiles, then the rest.
        xw = [[None] * n_xq for _ in range(NB)]

        def load_xq(lq):
            for wb in range(NB):
                t = cpool.tile([128, Lq * C_IN], BF16, name=f"xw_{wb}_{lq}")
                nc.sync.dma_start(
                    t.rearrange("w (l c) -> w l c", c=C_IN),
                    X_d[wb, :, lq * Lq:(lq + 1) * Lq, :],
                )
                xw[wb][lq] = t

        # h0 staging per pair: two v-half tiles [128, (l 2, v 256)] so the
        # two conv matmuls stream from DIFFERENT SBUF tiles (same-tile
        # moving streams do not run concurrently on the PE column strips).
        # Channel order is (G1, G2, G3, X) -- W^T rows rolled on the host --
        # so the 96-partition PSUM evac starts at partition 0 (quadrant
        # alignment rule); X^T pair-blocks are DMA'd into partitions 96:128
        # with XC_AHEAD pairs of lookahead.
        h0AB = {}

        def alloc_pair(p):
            if p >= cfg.L // 2:
                return
            a = h0sbA_pool.tile([128, 512], BF16, name="h0sA")
            b = h0sbB_pool.tile([128, 512], BF16, name="h0sB")
            nc.sync.dma_start(a[96:128, :], XC_d[p, 0])
            nc.sync.dma_start(b[96:128, :], XC_d[p, 1])
            h0AB[p] = (a, b)

        def emit_conv(p, lp):
            """Channel-mix conv for the l-pair (lp, lp+1) + bias.

            Four m=32 quarter-matmuls on distinct 32-column strips -- the
            only packing the PE runs concurrently (m=64 pairs serialize).
            Quarter q covers (vh, o-half) with output partitions 32q:32q+32.
            """
            a, b = h0AB.pop(p)
            cvp = cvps_pool.tile([128, 512], F32, name="cvp")
            for q, (rhs, oh) in enumerate(((a, 0), (a, 1), (b, 0), (b, 1))):
                nc.tensor.matmul(
                    cvp[32 * q:32 * (q + 1), :],
                    lhsT=w_t[:, 32 * oh:32 * (oh + 1)], rhs=rhs[:],
                    start=True, stop=True, tile_position=(0, 32 * q),
                    skip_group_check=True,
                )
            dst = out_sb[:, (lp % Lc) * 256:(lp % Lc + 2) * 256]
            if p % 2 == 0:
                nc.scalar.add(dst, cvp[:], b_t[:, 0:1])
            else:
                nc.vector.tensor_scalar_add(
                    out=dst, in0=cvp[:], scalar1=b_t[:, 0:1]
                )

        for k in range(GDEP):
            for wb in range(NB):
                if g_t[k][wb] is None:
                    t = cpool.tile([128, N], BF16, name=f"g{k}_{wb}")
                    nc.sync.dma_start(t[:], G_d[k, wb * 128:(wb + 1) * 128, :])
                    g_t[k][wb] = t
        # Upfront: X for the first 3 slices; the rest drips in one 128 KB
        # piece per 4 l so the DMA queues never develop a backlog that
        # delays the latency-critical X^T prefetches.
        for lq in range(3):
            load_xq(lq)
        for p in range(XC_AHEAD):
            alloc_pair(p)
        drip = [(wb, lq) for lq in range(3, n_xq) for wb in range(NB)]

        out_sb = out_pool.tile([128, 256 * Lc], BF16, name="out_sb")
        for l in range(cfg.L):
            p = l // 2
            if l % 4 == 0 and l // 4 < len(drip):
                wb_d, lq_d = drip[l // 4]
                t = cpool.tile(
                    [128, Lq * C_IN], BF16, name=f"xw_{wb_d}_{lq_d}"
                )
                nc.sync.dma_start(
                    t.rearrange("w (l c) -> w l c", c=C_IN),
                    X_d[wb_d, :, lq_d * Lq:(lq_d + 1) * Lq, :],
                )
                xw[wb_d][lq_d] = t
            h0p = h0ps_pool.tile([128, N], F32, name="h0p")
            for wb in range(NB):
                st = wb == 0
                sp = wb == NB - 1
                xl = xw[wb][l // Lq][:, (l % Lq) * C_IN:(l % Lq + 1) * C_IN]
                for k in range(GDEP):
                    nc.tensor.matmul(
                        h0p[32 * k:32 * (k + 1), :], lhsT=xl,
                        rhs=g_t[k][wb][:],
                        start=st, stop=sp, tile_position=(0, 32 * k),
                        skip_group_check=True,
                    )
            # evac hop channels into the (l 2, v 256) slots of each v-half
            a, b = h0AB[p]
            sl = slice((l % 2) * 256, (l % 2) * 256 + 256)
            if l % 2 == 0:
                nc.vector.tensor_copy(out=a[0:96, sl], in_=h0p[0:96, 0:256])
                nc.vector.tensor_copy(out=b[0:96, sl], in_=h0p[0:96, 256:512])
            else:
                nc.scalar.copy(a[0:96, sl], h0p[0:96, 0:256])
                nc.scalar.copy(b[0:96, sl], h0p[0:96, 256:512])

            if l % 2 == 1:
                alloc_pair(p + XC_AHEAD)
                if p > 0:
                    emit_conv(p - 1, l - 3)
                    if (l - 3) % Lc == Lc - 2:  # chunk complete -> store it
                        ch = (l - 3) // Lc
                        nc.sync.dma_start(
                            out_d[ch],
                            out_sb.rearrange("p (l v) -> p l v", v=256),
                        )
                        if ch + 1 < n_chunks:
                            out_sb = out_pool.tile(
                                [128, 256 * Lc], BF16, name="out_sb"
                            )
        emit_conv(cfg.L // 2 - 1, cfg.L - 2)
        nc.sync.dma_start(
            out_d[n_chunks - 1],
            out_sb.rearrange("p (l v) -> p l v", v=256),
        )


def build_nc(cfg: CFG):
    nc = bacc.Bacc("TRN2", target_bir_lowering=False, debug=False)
    n_chunks = cfg.L // cfg.Lc
    ins = {
        "xw": nc.dram_tensor("xw", [NB, 128, cfg.L, C_IN], BF16,
                             kind="ExternalInput").ap(),
        "xc": nc.dram_tensor("xc", [cfg.L // 2, 2, 32, 512], BF16,
                             kind="ExternalInput").ap(),
        "gt": nc.dram_tensor("gt", [GDEP, N, N], BF16,
                             kind="ExternalInput").ap(),
        "wt": nc.dram_tensor("wt", [128, C_OUT], BF16,
                             kind="ExternalInput").ap(),
        "bias2": nc.dram_tensor("bias2", [128, 1], F32,
                                kind="ExternalInput").ap(),
    }
    outs = {
        "out": nc.dram_tensor("out", [n_chunks, 128, cfg.Lc, 256], BF16,
                              kind="ExternalOutput").ap(),
    }
    with tile.TileContext(nc) as tc:
        body(nc, tc, outs, ins, cfg)
    nc.compile()
    return nc


def make_host_inputs(X, A, W, b):
    """Precompute all device operands on the host."""
    import ml_dtypes
    bf16 = ml_dtypes.bfloat16

    A = np.asarray(A, np.float64)
    n = A.shape[0]
    An = A + np.eye(n)
    An = An / An.sum(axis=1, keepdims=True)
    As = (1.0 - BETA) * An
    eye = np.eye(n)
    G = []
    gk = eye
    for _ in range(GDEP):
        gk = As @ gk + BETA * eye
        G.append(gk)
    GT = np.stack([g.T for g in G]).astype(bf16)  # [GDEP, N, N]
    # W^T rows rolled so the channel order is (G1, G2, G3, X), matching the
    # device-side concat layout (hops in PSUM partitions 0:96, X DMA'd into
    # 96:128).
    WT = np.roll(np.asarray(W, np.float64).T, -C_IN, axis=0)
    WT = np.ascontiguousarray(WT.astype(bf16))
    b = np.asarray(b, np.float32)
    b2 = np.ascontiguousarray(np.concatenate([b, b]).reshape(128, 1))

    # X [B, C_IN, N, L] f32 -> per core [NB, 128, L, C_IN] bf16 (lhsT layout)
    X = np.asarray(X)
    XW = np.ascontiguousarray(X.transpose(0, 2, 3, 1)).astype(bf16)
    XW = XW.reshape(B, NB, 128, L, C_IN)
    # X^T pair-blocks [L//2, vh 2, c 32, (l 2, v 256)] bf16 for direct DMA
    # into the h0 staging tiles' partitions 96:128.
    XC = X.reshape(B, C_IN, 2, 256, L // 2, 2).transpose(0, 4, 2, 1, 5, 3)
    XC = np.ascontiguousarray(XC).astype(bf16).reshape(B, L // 2, 2, 32, 512)
    return XW, XC, GT, WT, b2


_NC_CACHE = {}


def run_on_hw(X, A, W, b, cfg=None, trace=False, **spmd_kwargs):
    XW, XC, GT, WT, b2 = make_host_inputs(X, A, W, b)
    if cfg is None:
        cfg = CFG()
    key = (cfg.L, cfg.Lc, cfg.Lq, cfg.n_warm)
    if key not in _NC_CACHE:
        _NC_CACHE[key] = build_nc(cfg)
    nc = _NC_CACHE[key]
    in_maps = [
        {"xw": XW[i], "xc": XC[i], "gt": GT, "wt": WT, "bias2": b2}
        for i in range(B)
    ]
    res = bass_utils.run_bass_kernel_spmd(
        nc, in_maps, core_ids=list(range(B)), trace=trace, **spmd_kwargs
    )
    # out_dev [n_chunks, 128=(vh,o), Lc, 256=v] bf16
    #   -> out [C_OUT, N, L] f32  via (o, vh, v, ch, l)
    n_chunks = cfg.L // cfg.Lc
    outs = []
    for i in range(B):
        o = np.asarray(res.results[i]["out"])
        o = o.reshape(n_chunks, 2, C_OUT, cfg.Lc, 256)
        o = o.transpose(2, 1, 4, 0, 3).reshape(C_OUT, N, cfg.L)
        outs.append(o.astype(np.float32))
    return np.stack(outs), res


def kernel(X, A, W, b):
    return run_on_hw(X, A, W, b)[0]


if __name__ == "__main__":
    rng = np.random.default_rng(0)
    X = rng.standard_normal((B, C_IN, N, L), dtype=np.float32)
    A = rng.random((N, N), dtype=np.float32)
    W = rng.standard_normal((C_OUT, (GDEP + 1) * C_IN), dtype=np.float32) * 0.1
    b = rng.random(C_OUT, dtype=np.float32)
    out = kernel(X, A, W, b)
    print("out", out.shape, out.dtype, float(np.abs(out).mean()))


# revision 32
# speedup vs baseline: 1.0997x; 1.0997x over previous
"""MixProp GNN message-passing kernel for 8 TRN2 NeuronCores.

Reference computation (per batch element b):
    A_n = row_normalize(A + I)
    H_0 = X;  H_k = beta*X + (1-beta) * A_n @_nodes H_{k-1}   (k=1..3)
    out = W @_channels concat([H_0..H_3]) + bias

Kernel strategy (v3):
  - Data-parallel over batch: B=8 batch elements -> 8 cores, no collectives.
  - Host precomputes G_k s.t. H_k = G_k @ X, pre-casts operands to bf16 and
    pre-transposes X into the lhsT layout [wb, w, l, c]: the device does no
    layout work on X.
  - Per seq position l: per 128-node block, 3 column-packed matmuls (G1..G3)
    build PSUM H0[(hop,ch), v]; the column groups stream concurrently so
    each l costs ~4x512 moving columns (~850 ns warm).
  - The X part of the concat (H_0 = X itself, channels 0:32 of the conv)
    never touches the PE: the host supplies X^T pair-blocks and they are
    DMA'd straight into partitions 0:32 of the staging tile. (v3 computed
    X^T on the PE via identity matmuls; the tile scheduler sank those into
    the conv region where ~23 short instructions serialized at the ~50 ns
    dispatch floor, costing ~600 ns per pair.)
  - Seq positions are paired for the channel conv: both l's H0 go into one
    [128, 1024] SBUF tile (v-half-major), the conv is 2 matmuls of n=512
    into one PSUM bank, evacuated (+bias) in a single op.
  - PSUM->SBUF evacuation alternates DVE / Scalar engine per l.
  - Output staged bf16 as [(vh,o), (l, v)] per 32-l chunk, stored to a
    chunk-major DRAM layout (16 KB contiguous per partition); the host
    reassembles [64, 512, 256] f32 (bf16 output rounding ~0.4% rel, well
    inside the 2e-2 gate).
  - DMA order: first G tile lands in ~1 us and feeds ~48 HAM-warmup
    matmuls; X arrives in 64-l slices, first-needed first, so real compute
    starts ~12 us in instead of waiting for the full 10 MB fill.
"""

import sys

sys.path.insert(0, "/opt/trn_rl_repo")

import numpy as np

import concourse.bass as bass
import concourse.bacc as bacc
import concourse.mybir as mybir
from concourse import tile
from concourse import bass_utils

GDEP = 3
BETA = 0.05
C_IN = 32
C_OUT = 64
N = 512
B = 8
L = 256
NB = N // 128  # node blocks of 128

F32 = mybir.dt.float32
BF16 = mybir.dt.bfloat16


class CFG:
    def __init__(self, L=L, Lc=16, Lq=64, n_warm=48, xc_ahead=16):
        assert L % Lc == 0 and L % Lq == 0
        self.L = L
        self.Lc = Lc      # output store chunk
        self.Lq = Lq      # X load slice
        self.n_warm = n_warm
        self.xc_ahead = xc_ahead  # pairs of X^T DMA prefetch lookahead


def body(nc, tc, outs, ins, cfg: CFG):
    """Emit the per-core program. ins/outs are dicts of DRAM APs."""
    X_d = ins["xw"]         # [NB, 128, L, C_IN] bf16  pre-transposed lhsT
    XC_d = ins["xc"]        # [L//2, 2, 32, 512] bf16 X^T pair-blocks (vh)(l,v)
    G_d = ins["gt"]         # [GDEP, N, N] bf16  G_k^T
    W_d = ins["wt"]         # [128, C_OUT] bf16  W^T
    b_d = ins["bias2"]      # [128, 1]     f32   bias duplicated for (vh, o)
    out_d = outs["out"]     # [n_chunks, 128, Lc, 256] bf16 chunk-major

    Lc, Lq = cfg.Lc, cfg.Lq
    n_chunks = cfg.L // Lc
    n_xq = cfg.L // Lq

    XC_AHEAD = cfg.xc_ahead

    with (
        tc.tile_pool(name="const", bufs=1) as cpool,
        tc.tile_pool(name="h0sb", bufs=XC_AHEAD + 2) as h0sb_pool,
        tc.tile_pool(name="outsb", bufs=2) as out_pool,
        tc.tile_pool(name="h0ps", bufs=3, space="PSUM") as h0ps_pool,
        tc.tile_pool(name="cvps", bufs=3, space="PSUM") as cvps_pool,
        tc.tile_pool(name="wmps", bufs=1, space="PSUM") as wm_pool,
    ):
        # ---- DMA order is load-bearing: g00 feeds the warmup matmuls ----
        g_t = [[None] * NB for _ in range(GDEP)]
        g_t[0][0] = cpool.tile([128, N], BF16, name="g0_0")
        nc.sync.dma_start(g_t[0][0][:], G_d[0, 0:128, :])

        w_t = cpool.tile([128, C_OUT], BF16, name="w_t")
        nc.sync.dma_start(w_t[:], W_d[:])
        b_t = cpool.tile([128, 1], F32, name="b_t")
        nc.sync.dma_start(b_t[:], b_d[:])

        # HAM warmup on the PE while the bulk DMAs land.
        wm = wm_pool.tile([128, N], F32, name="wm")
        for _ in range(cfg.n_warm):
            nc.tensor.matmul(
                wm[:], lhsT=g_t[0][0][:, 0:128], rhs=g_t[0][0][:],
                start=True, stop=True,
            )

        # X slices needed first, then the remaining G tiles, then the rest.
        xw = [[None] * n_xq for _ in range(NB)]

        def load_xq(lq):
            for wb in range(NB):
                t = cpool.tile([128, Lq * C_IN], BF16, name=f"xw_{wb}_{lq}")
                nc.sync.dma_start(
                    t.rearrange("w (l c) -> w l c", c=C_IN),
                    X_d[wb, :, lq * Lq:(lq + 1) * Lq, :],
                )
                xw[wb][lq] = t

        # h0 staging per pair: one [128, (vh 2, l 2, v 256)] tile. Channel
        # order is (G1, G2, G3, X) -- W^T rows rolled on the host -- so the
        # 96-partition PSUM evac starts at partition 0 (quadrant alignment
        # rule); X^T pair-blocks are DMA'd into partitions 96:128 with
        # XC_AHEAD pairs of lookahead.
        h0S = {}

        def alloc_pair(p):
            if p >= cfg.L // 2:
                return
            s = h0sb_pool.tile([128, 1024], BF16, name="h0s2")
            nc.sync.dma_start(s[96:128, :], XC_d[p])
            h0S[p] = s

        def emit_conv(p, lp):
            """Channel-mix conv for the l-pair (lp, lp+1) + bias.

            Four m=32 quarter-matmuls on distinct 32-column strips -- the
            only packing the PE runs concurrently (m=64 pairs serialize).
            Quarter q covers (vh, o-half) with output partitions 32q:32q+32.
            """
            s = h0S.pop(p)
            cvp = cvps_pool.tile([128, 512], F32, name="cvp")
            for q in range(4):
                vh, oh = q // 2, q % 2
                nc.tensor.matmul(
                    cvp[32 * q:32 * (q + 1), :],
                    lhsT=w_t[:, 32 * oh:32 * (oh + 1)],
                    rhs=s[:, 512 * vh:512 * (vh + 1)],
                    start=True, stop=True, tile_position=(0, 32 * q),
                    skip_group_check=True,
                )
            dst = out_sb[:, (lp % Lc) * 256:(lp % Lc + 2) * 256]
            if p % 2 == 0:
                nc.scalar.add(dst, cvp[:], b_t[:, 0:1])
            else:
                nc.vector.tensor_scalar_add(
                    out=dst, in0=cvp[:], scalar1=b_t[:, 0:1]
                )

        for k in range(GDEP):
            for wb in range(NB):
                if g_t[k][wb] is None:
                    t = cpool.tile([128, N], BF16, name=f"g{k}_{wb}")
                    nc.sync.dma_start(t[:], G_d[k, wb * 128:(wb + 1) * 128, :])
                    g_t[k][wb] = t
        # Upfront: latency-critical X^T prefetches first, then X for the
        # first slice; the rest drips in one 512 KB piece per 16 l so the
        # DMA queues never develop a backlog that delays the prefetches.
        for p in range(XC_AHEAD):
            alloc_pair(p)
        load_xq(0)
        drip = [(wb, lq) for lq in range(1, n_xq) for wb in range(NB)]

        out_sb = out_pool.tile([128, 256 * Lc], BF16, name="out_sb")
        for l in range(cfg.L):
            p = l // 2
            if l % 16 == 0 and l // 16 < len(drip):
                wb_d, lq_d = drip[l // 16]
                t = cpool.tile(
                    [128, Lq * C_IN], BF16, name=f"xw_{wb_d}_{lq_d}"
                )
                nc.sync.dma_start(
                    t.rearrange("w (l c) -> w l c", c=C_IN),
                    X_d[wb_d, :, lq_d * Lq:(lq_d + 1) * Lq, :],
                )
                xw[wb_d][lq_d] = t
            h0p = h0ps_pool.tile([128, N], F32, name="h0p")
            for wb in range(NB):
                st = wb == 0
                sp = wb == NB - 1
                xl = xw[wb][l // Lq][:, (l % Lq) * C_IN:(l % Lq + 1) * C_IN]
                for k in range(GDEP):
                    nc.tensor.matmul(
                        h0p[32 * k:32 * (k + 1), :], lhsT=xl,
                        rhs=g_t[k][wb][:],
                        start=st, stop=sp, tile_position=(0, 32 * k),
                        skip_group_check=True,
                    )
            # evac hop channels into the (vh 2, l 2, v 256) slots
            dst = h0S[p].rearrange("p (vh l v) -> p vh l v", vh=2, l=2)[
                0:96, :, l % 2, :
            ]
            if l % 2 == 0:
                nc.vector.tensor_copy(out=dst, in_=h0p[0:96, :])
            else:
                nc.scalar.copy(dst, h0p[0:96, :])

            if l % 2 == 1:
                alloc_pair(p + XC_AHEAD)
                if p > 0:
                    emit_conv(p - 1, l - 3)
                    if (l - 3) % Lc == Lc - 2:  # chunk complete -> store it
                        ch = (l - 3) // Lc
                        nc.sync.dma_start(
                            out_d[ch],
                            out_sb.rearrange("p (l v) -> p l v", v=256),
                        )
                        if ch + 1 < n_chunks:
                            out_sb = out_pool.tile(
                                [128, 256 * Lc], BF16, name="out_sb"
                            )
        emit_conv(cfg.L // 2 - 1, cfg.L - 2)
        nc.sync.dma_start(
            out_d[n_chunks - 1],
            out_sb.rearrange("p (l v) -> p l v", v=256),
        )


def build_nc(cfg: CFG):
    nc = bacc.Bacc("TRN2", target_bir_lowering=False, debug=False)
    n_chunks = cfg.L // cfg.Lc
    ins = {
        "xw": nc.dram_tensor("xw", [NB, 128, cfg.L, C_IN], BF16,
                             kind="ExternalInput").ap(),
        "xc": nc.dram_tensor("xc", [cfg.L // 2, 32, 1024], BF16,
                             kind="ExternalInput").ap(),
        "gt": nc.dram_tensor("gt", [GDEP, N, N], BF16,
                             kind="ExternalInput").ap(),
        "wt": nc.dram_tensor("wt", [128, C_OUT], BF16,
                             kind="ExternalInput").ap(),
        "bias2": nc.dram_tensor("bias2", [128, 1], F32,
                                kind="ExternalInput").ap(),
    }
    outs = {
        "out": nc.dram_tensor("out", [n_chunks, 128, cfg.Lc, 256], BF16,
                              kind="ExternalOutput").ap(),
    }
    with tile.TileContext(nc) as tc:
        body(nc, tc, outs, ins, cfg)
    nc.compile()
    return nc


def make_host_inputs(X, A, W, b):
    """Precompute all device operands on the host."""
    import ml_dtypes
    bf16 = ml_dtypes.bfloat16

    A = np.asarray(A, np.float64)
    n = A.shape[0]
    An = A + np.eye(n)
    An = An / An.sum(axis=1, keepdims=True)
    As = (1.0 - BETA) * An
    eye = np.eye(n)
    G = []
    gk = eye
    for _ in range(GDEP):
        gk = As @ gk + BETA * eye
        G.append(gk)
    GT = np.stack([g.T for g in G]).astype(bf16)  # [GDEP, N, N]
    # W^T rows rolled so the channel order is (G1, G2, G3, X), matching the
    # device-side concat layout (hops in PSUM partitions 0:96, X DMA'd into
    # 96:128).
    WT = np.roll(np.asarray(W, np.float64).T, -C_IN, axis=0)
    WT = np.ascontiguousarray(WT.astype(bf16))
    b = np.asarray(b, np.float32)
    b2 = np.ascontiguousarray(np.concatenate([b, b]).reshape(128, 1))

    # X [B, C_IN, N, L] f32 -> per core [NB, 128, L, C_IN] bf16 (lhsT layout)
    X = np.asarray(X)
    XW = np.ascontiguousarray(X.transpose(0, 2, 3, 1)).astype(bf16)
    XW = XW.reshape(B, NB, 128, L, C_IN)
    # X^T pair-blocks [L//2, c 32, (vh 2, l 2, v 256)] bf16 for direct DMA
    # into the h0 staging tiles' partitions 96:128.
    XC = X.reshape(B, C_IN, 2, 256, L // 2, 2).transpose(0, 4, 1, 2, 5, 3)
    XC = np.ascontiguousarray(XC).astype(bf16).reshape(B, L // 2, 32, 1024)
    return XW, XC, GT, WT, b2


_NC_CACHE = {}


def run_on_hw(X, A, W, b, cfg=None, trace=False, **spmd_kwargs):
    XW, XC, GT, WT, b2 = make_host_inputs(X, A, W, b)
    if cfg is None:
        cfg = CFG()
    key = (cfg.L, cfg.Lc, cfg.Lq, cfg.n_warm)
    if key not in _NC_CACHE:
        _NC_CACHE[key] = build_nc(cfg)
    nc = _NC_CACHE[key]
    in_maps = [
        {"xw": XW[i], "xc": XC[i], "gt": GT, "wt": WT, "bias2": b2}
        for i in range(B)
    ]
    res = bass_utils.run_bass_kernel_spmd(
        nc, in_maps, core_ids=list(range(B)), trace=trace, **spmd_kwargs
    )
    # out_dev [n_chunks, 128=(vh,o), Lc, 256=v] bf16
    #   -> out [C_OUT, N, L] f32  via (o, vh, v, ch, l)
    n_chunks = cfg.L // cfg.Lc
    outs = []
    for i in range(B):
        o = np.asarray(res.results[i]["out"])
        o = o.reshape(n_chunks, 2, C_OUT, cfg.Lc, 256)
        o = o.transpose(2, 1, 4, 0, 3).reshape(C_OUT, N, cfg.L)
        outs.append(o.astype(np.float32))
    return np.stack(outs), res


def kernel(X, A, W, b):
    return run_on_hw(X, A, W, b)[0]


if __name__ == "__main__":
    rng = np.random.default_rng(0)
    X = rng.standard_normal((B, C_IN, N, L), dtype=np.float32)
    A = rng.random((N, N), dtype=np.float32)
    W = rng.standard_normal((C_OUT, (GDEP + 1) * C_IN), dtype=np.float32) * 0.1
    b = rng.random(C_OUT, dtype=np.float32)
    out = kernel(X, A, W, b)
    print("out", out.shape, out.dtype, float(np.abs(out).mean()))


# revision 37
# speedup vs baseline: 1.1068x; 1.0065x over previous
"""MixProp GNN message-passing kernel for 8 TRN2 NeuronCores.

Reference computation (per batch element b):
    A_n = row_normalize(A + I)
    H_0 = X;  H_k = beta*X + (1-beta) * A_n @_nodes H_{k-1}   (k=1..3)
    out = W @_channels concat([H_0..H_3]) + bias

Kernel strategy (v3):
  - Data-parallel over batch: B=8 batch elements -> 8 cores, no collectives.
  - Host precomputes G_k s.t. H_k = G_k @ X, pre-casts operands to bf16 and
    pre-transposes X into the lhsT layout [wb, w, l, c]: the device does no
    layout work on X.
  - Per seq position l: per 128-node block, 3 column-packed matmuls (G1..G3)
    build PSUM H0[(hop,ch), v]; the column groups stream concurrently so
    each l costs ~4x512 moving columns (~850 ns warm).
  - The X part of the concat (H_0 = X itself, channels 0:32 of the conv)
    never touches the PE: the host supplies X^T pair-blocks and they are
    DMA'd straight into partitions 0:32 of the staging tile. (v3 computed
    X^T on the PE via identity matmuls; the tile scheduler sank those into
    the conv region where ~23 short instructions serialized at the ~50 ns
    dispatch floor, costing ~600 ns per pair.)
  - Seq positions are paired for the channel conv: both l's H0 go into one
    [128, 1024] SBUF tile (v-half-major), the conv is 2 matmuls of n=512
    into one PSUM bank, evacuated (+bias) in a single op.
  - PSUM->SBUF evacuation alternates DVE / Scalar engine per l.
  - Output staged bf16 as [(vh,o), (l, v)] per 32-l chunk, stored to a
    chunk-major DRAM layout (16 KB contiguous per partition); the host
    reassembles [64, 512, 256] f32 (bf16 output rounding ~0.4% rel, well
    inside the 2e-2 gate).
  - DMA order: first G tile lands in ~1 us and feeds ~48 HAM-warmup
    matmuls; X arrives in 64-l slices, first-needed first, so real compute
    starts ~12 us in instead of waiting for the full 10 MB fill.
"""

import sys

sys.path.insert(0, "/opt/trn_rl_repo")

import numpy as np

import concourse.bass as bass
import concourse.bacc as bacc
import concourse.mybir as mybir
from concourse import tile
from concourse import bass_utils

GDEP = 3
BETA = 0.05
C_IN = 32
C_OUT = 64
N = 512
B = 8
L = 256
NB = N // 128  # node blocks of 128

F32 = mybir.dt.float32
BF16 = mybir.dt.bfloat16


class CFG:
    def __init__(self, L=L, Lc=16, Lq=64, n_warm=48, xc_ahead=16):
        assert L % Lc == 0 and L % Lq == 0
        self.L = L
        self.Lc = Lc      # output store chunk
        self.Lq = Lq      # X load slice
        self.n_warm = n_warm
        self.xc_ahead = xc_ahead  # pairs of X^T DMA prefetch lookahead


def body(nc, tc, outs, ins, cfg: CFG):
    """Emit the per-core program. ins/outs are dicts of DRAM APs."""
    X_d = ins["xw"]         # [NB, 128, L, C_IN] bf16  pre-transposed lhsT
    XC_d = ins["xc"]        # [L//2, 32, 1024] bf16  X^T pair-blocks (vh,l,v)
    G_d = ins["gt"]         # [128, GDEP, NB, 512] bf16  w-major G_k^T blocks
    W_d = ins["wt"]         # [128, C_OUT] bf16  W^T
    b_d = ins["bias2"]      # [128, 1]     f32   bias duplicated for (vh, o)
    out_d = outs["out"]     # [n_chunks, 128, Lc, 256] bf16 chunk-major

    Lc, Lq = cfg.Lc, cfg.Lq
    n_chunks = cfg.L // Lc
    n_xq = cfg.L // Lq

    XC_AHEAD = cfg.xc_ahead

    with (
        tc.tile_pool(name="const", bufs=1) as cpool,
        tc.tile_pool(name="h0sb", bufs=XC_AHEAD + 2) as h0sb_pool,
        tc.tile_pool(name="outsb", bufs=2) as out_pool,
        tc.tile_pool(name="h0ps", bufs=3, space="PSUM") as h0ps_pool,
        tc.tile_pool(name="cvps", bufs=3, space="PSUM") as cvps_pool,
        tc.tile_pool(name="wmps", bufs=1, space="PSUM") as wm_pool,
    ):
        # ---- DMA order is load-bearing: g00 feeds the warmup matmuls ----
        g00 = cpool.tile([128, N], BF16, name="g00")
        nc.sync.dma_start(g00[:], G_d[:, 0, 0, :])

        w_t = cpool.tile([128, C_OUT], BF16, name="w_t")
        nc.sync.dma_start(w_t[:], W_d[:])
        b_t = cpool.tile([128, 1], F32, name="b_t")
        nc.sync.dma_start(b_t[:], b_d[:])
        # all 12 G block tiles in ONE w-major DMA (128 x 12 KB descriptors)
        g_all = cpool.tile([128, GDEP * NB * N], BF16, name="g_all")
        nc.sync.dma_start(
            g_all.rearrange("w (k b v) -> w k b v", k=GDEP, b=NB), G_d
        )
        g_t = [
            [
                g_all[:, (k * NB + wb) * N:(k * NB + wb + 1) * N]
                for wb in range(NB)
            ]
            for k in range(GDEP)
        ]

        # HAM warmup on the PE while the bulk DMAs land; a dummy Scalar
        # activation pulls the ACT table load (~2.7 us) into the fill phase.
        sc_warm = cpool.tile([128, 1], F32, name="sc_warm")
        nc.scalar.add(sc_warm[:], b_t[:], 0.0)
        wm = wm_pool.tile([128, N], F32, name="wm")
        for _ in range(cfg.n_warm):
            nc.tensor.matmul(
                wm[:], lhsT=g00[:, 0:128], rhs=g00[:],
                start=True, stop=True,
            )

        # X slices needed first, then the remaining G tiles, then the rest.
        xw = [[None] * n_xq for _ in range(NB)]

        def load_xq(lq):
            for wb in range(NB):
                t = cpool.tile([128, Lq * C_IN], BF16, name=f"xw_{wb}_{lq}")
                nc.sync.dma_start(
                    t.rearrange("w (l c) -> w l c", c=C_IN),
                    X_d[wb, :, lq * Lq:(lq + 1) * Lq, :],
                )
                xw[wb][lq] = t

        # h0 staging per pair: one [128, (vh 2, l 2, v 256)] tile. Channel
        # order is (G1, G2, G3, X) -- W^T rows rolled on the host -- so the
        # 96-partition PSUM evac starts at partition 0 (quadrant alignment
        # rule); X^T pair-blocks are DMA'd into partitions 96:128 with
        # XC_AHEAD pairs of lookahead.
        h0S = {}

        def alloc_pair(p):
            if p >= cfg.L // 2:
                return
            s = h0sb_pool.tile([128, 1024], BF16, name="h0s2")
            nc.sync.dma_start(s[96:128, :], XC_d[p])
            h0S[p] = s

        def emit_conv(p, lp):
            """Channel-mix conv for the l-pair (lp, lp+1) + bias.

            Four m=32 quarter-matmuls on distinct 32-column strips -- the
            only packing the PE runs concurrently (m=64 pairs serialize).
            Quarter q covers (vh, o-half) with output partitions 32q:32q+32.
            """
            s = h0S.pop(p)
            cvp = cvps_pool.tile([128, 512], F32, name="cvp")
            for q in range(4):
                vh, oh = q // 2, q % 2
                nc.tensor.matmul(
                    cvp[32 * q:32 * (q + 1), :],
                    lhsT=w_t[:, 32 * oh:32 * (oh + 1)],
                    rhs=s[:, 512 * vh:512 * (vh + 1)],
                    start=True, stop=True, tile_position=(0, 32 * q),
                    skip_group_check=True,
                )
            dst = out_sb[:, (lp % Lc) * 256:(lp % Lc + 2) * 256]
            if p % 2 == 0:
                nc.scalar.add(dst, cvp[:], b_t[:, 0:1])
            else:
                nc.vector.tensor_scalar_add(
                    out=dst, in0=cvp[:], scalar1=b_t[:, 0:1]
                )

        # Upfront: latency-critical X^T prefetches first, then X for the
        # first slice; the rest drips in one 512 KB piece per 16 l so the
        # DMA queues never develop a backlog that delays the prefetches.
        for p in range(XC_AHEAD):
            alloc_pair(p)
        load_xq(0)
        drip = [(wb, lq) for lq in range(1, n_xq) for wb in range(NB)]

        out_sb = out_pool.tile([128, 256 * Lc], BF16, name="out_sb")
        for l in range(cfg.L):
            p = l // 2
            if l % 16 == 0 and l // 16 < len(drip):
                wb_d, lq_d = drip[l // 16]
                t = cpool.tile(
                    [128, Lq * C_IN], BF16, name=f"xw_{wb_d}_{lq_d}"
                )
                nc.sync.dma_start(
                    t.rearrange("w (l c) -> w l c", c=C_IN),
                    X_d[wb_d, :, lq_d * Lq:(lq_d + 1) * Lq, :],
                )
                xw[wb_d][lq_d] = t
            h0p = h0ps_pool.tile([128, N], F32, name="h0p")
            for wb in range(NB):
                st = wb == 0
                sp = wb == NB - 1
                xl = xw[wb][l // Lq][:, (l % Lq) * C_IN:(l % Lq + 1) * C_IN]
                for k in range(GDEP):
                    nc.tensor.matmul(
                        h0p[32 * k:32 * (k + 1), :], lhsT=xl,
                        rhs=g_t[k][wb][:],
                        start=st, stop=sp, tile_position=(0, 32 * k),
                        skip_group_check=True,
                    )
            # evac hop channels into the (vh 2, l 2, v 256) slots
            dst = h0S[p].rearrange("p (vh l v) -> p vh l v", vh=2, l=2)[
                0:96, :, l % 2, :
            ]
            if l % 2 == 0:
                nc.vector.tensor_copy(out=dst, in_=h0p[0:96, :])
            else:
                nc.scalar.copy(dst, h0p[0:96, :])

            if l % 2 == 1:
                alloc_pair(p + XC_AHEAD)
                if p > 0:
                    emit_conv(p - 1, l - 3)
                    if (l - 3) % Lc == Lc - 2:  # chunk complete -> store it
                        ch = (l - 3) // Lc
                        nc.sync.dma_start(
                            out_d[ch],
                            out_sb.rearrange("p (l v) -> p l v", v=256),
                        )
                        if ch + 1 < n_chunks:
                            out_sb = out_pool.tile(
                                [128, 256 * Lc], BF16, name="out_sb"
                            )
        emit_conv(cfg.L // 2 - 1, cfg.L - 2)
        nc.sync.dma_start(
            out_d[n_chunks - 1],
            out_sb.rearrange("p (l v) -> p l v", v=256),
        )


def build_nc(cfg: CFG):
    nc = bacc.Bacc("TRN2", target_bir_lowering=False, debug=False)
    n_chunks = cfg.L // cfg.Lc
    ins = {
        "xw": nc.dram_tensor("xw", [NB, 128, cfg.L, C_IN], BF16,
                             kind="ExternalInput").ap(),
        "xc": nc.dram_tensor("xc", [cfg.L // 2, 32, 1024], BF16,
                             kind="ExternalInput").ap(),
        "gt": nc.dram_tensor("gt", [128, GDEP, NB, 512], BF16,
                             kind="ExternalInput").ap(),
        "wt": nc.dram_tensor("wt", [128, C_OUT], BF16,
                             kind="ExternalInput").ap(),
        "bias2": nc.dram_tensor("bias2", [128, 1], F32,
                                kind="ExternalInput").ap(),
    }
    outs = {
        "out": nc.dram_tensor("out", [n_chunks, 128, cfg.Lc, 256], BF16,
                              kind="ExternalOutput").ap(),
    }
    with tile.TileContext(nc) as tc:
        body(nc, tc, outs, ins, cfg)
    nc.compile()
    return nc


def make_host_inputs(X, A, W, b):
    """Precompute all device operands on the host."""
    import ml_dtypes
    bf16 = ml_dtypes.bfloat16

    A = np.asarray(A, np.float64)
    n = A.shape[0]
    An = A + np.eye(n)
    An = An / An.sum(axis=1, keepdims=True)
    As = (1.0 - BETA) * An
    eye = np.eye(n)
    G = []
    gk = eye
    for _ in range(GDEP):
        gk = As @ gk + BETA * eye
        G.append(gk)
    GT = np.stack([g.T for g in G]).astype(bf16)  # [GDEP, N, N]
    # w-major G blocks [w 128, k GDEP, wb NB, v 512] for one big load
    GT = np.ascontiguousarray(
        GT.reshape(GDEP, NB, 128, N).transpose(2, 0, 1, 3)
    )
    # W^T rows rolled so the channel order is (G1, G2, G3, X), matching the
    # device-side concat layout (hops in PSUM partitions 0:96, X DMA'd into
    # 96:128).
    WT = np.roll(np.asarray(W, np.float64).T, -C_IN, axis=0)
    WT = np.ascontiguousarray(WT.astype(bf16))
    b = np.asarray(b, np.float32)
    b2 = np.ascontiguousarray(np.concatenate([b, b]).reshape(128, 1))

    # X [B, C_IN, N, L] f32 -> per core [NB, 128, L, C_IN] bf16 (lhsT layout)
    X = np.asarray(X)
    XW = np.ascontiguousarray(X.transpose(0, 2, 3, 1)).astype(bf16)
    XW = XW.reshape(B, NB, 128, L, C_IN)
    # X^T pair-blocks [L//2, c 32, (vh 2, l 2, v 256)] bf16 for direct DMA
    # into the h0 staging tiles' partitions 96:128.
    XC = X.reshape(B, C_IN, 2, 256, L // 2, 2).transpose(0, 4, 1, 2, 5, 3)
    XC = np.ascontiguousarray(XC).astype(bf16).reshape(B, L // 2, 32, 1024)
    return XW, XC, GT, WT, b2


_NC_CACHE = {}


def run_on_hw(X, A, W, b, cfg=None, trace=False, **spmd_kwargs):
    XW, XC, GT, WT, b2 = make_host_inputs(X, A, W, b)
    if cfg is None:
        cfg = CFG()
    key = (cfg.L, cfg.Lc, cfg.Lq, cfg.n_warm)
    if key not in _NC_CACHE:
        _NC_CACHE[key] = build_nc(cfg)
    nc = _NC_CACHE[key]
    in_maps = [
        {"xw": XW[i], "xc": XC[i], "gt": GT, "wt": WT, "bias2": b2}
        for i in range(B)
    ]
    res = bass_utils.run_bass_kernel_spmd(
        nc, in_maps, core_ids=list(range(B)), trace=trace, **spmd_kwargs
    )
    # out_dev [n_chunks, 128=(vh,o), Lc, 256=v] bf16
    #   -> out [C_OUT, N, L] f32  via (o, vh, v, ch, l)
    n_chunks = cfg.L // cfg.Lc
    outs = []
    for i in range(B):
        o = np.asarray(res.results[i]["out"])
        o = o.reshape(n_chunks, 2, C_OUT, cfg.Lc, 256)
        o = o.transpose(2, 1, 4, 0, 3).reshape(C_OUT, N, cfg.L)
        outs.append(o.astype(np.float32))
    return np.stack(outs), res


def kernel(X, A, W, b):
    return run_on_hw(X, A, W, b)[0]


if __name__ == "__main__":
    rng = np.random.default_rng(0)
    X = rng.standard_normal((B, C_IN, N, L), dtype=np.float32)
    A = rng.random((N, N), dtype=np.float32)
    W = rng.standard_normal((C_OUT, (GDEP + 1) * C_IN), dtype=np.float32) * 0.1
    b = rng.random(C_OUT, dtype=np.float32)
    out = kernel(X, A, W, b)
    print("out", out.shape, out.dtype, float(np.abs(out).mean()))


# revision 44
# speedup vs baseline: 1.1230x; 1.0147x over previous
"""MixProp GNN message-passing kernel for 8 TRN2 NeuronCores.

Reference computation (per batch element b):
    A_n = row_normalize(A + I)
    H_0 = X;  H_k = beta*X + (1-beta) * A_n @_nodes H_{k-1}   (k=1..3)
    out = W @_channels concat([H_0..H_3]) + bias

Kernel strategy (v3):
  - Data-parallel over batch: B=8 batch elements -> 8 cores, no collectives.
  - Host precomputes G_k s.t. H_k = G_k @ X, pre-casts operands to bf16 and
    pre-transposes X into the lhsT layout [wb, w, l, c]: the device does no
    layout work on X.
  - Per seq position l: per 128-node block, 3 column-packed matmuls (G1..G3)
    build PSUM H0[(hop,ch), v]; the column groups stream concurrently so
    each l costs ~4x512 moving columns (~850 ns warm).
  - The X part of the concat (H_0 = X itself, channels 0:32 of the conv)
    never touches the PE: the host supplies X^T pair-blocks and they are
    DMA'd straight into partitions 0:32 of the staging tile. (v3 computed
    X^T on the PE via identity matmuls; the tile scheduler sank those into
    the conv region where ~23 short instructions serialized at the ~50 ns
    dispatch floor, costing ~600 ns per pair.)
  - Seq positions are paired for the channel conv: both l's H0 go into one
    [128, 1024] SBUF tile (v-half-major), the conv is 2 matmuls of n=512
    into one PSUM bank, evacuated (+bias) in a single op.
  - PSUM->SBUF evacuation alternates DVE / Scalar engine per l.
  - Output staged bf16 as [(vh,o), (l, v)] per 32-l chunk, stored to a
    chunk-major DRAM layout (16 KB contiguous per partition); the host
    reassembles [64, 512, 256] f32 (bf16 output rounding ~0.4% rel, well
    inside the 2e-2 gate).
  - DMA order: first G tile lands in ~1 us and feeds ~48 HAM-warmup
    matmuls; X arrives in 64-l slices, first-needed first, so real compute
    starts ~12 us in instead of waiting for the full 10 MB fill.
"""

import sys

sys.path.insert(0, "/opt/trn_rl_repo")

import numpy as np

import concourse.bass as bass
import concourse.bacc as bacc
import concourse.mybir as mybir
from concourse import tile
from concourse import bass_utils

GDEP = 3
BETA = 0.05
C_IN = 32
C_OUT = 64
N = 512
B = 8
L = 256
NB = N // 128  # node blocks of 128

F32 = mybir.dt.float32
BF16 = mybir.dt.bfloat16


class CFG:
    def __init__(self, L=L, Lc=16, Lq=64, n_warm=72, xc_ahead=16):
        assert L % Lc == 0 and L % Lq == 0
        self.L = L
        self.Lc = Lc      # output store chunk
        self.Lq = Lq      # X load slice
        self.n_warm = n_warm
        self.xc_ahead = xc_ahead  # pairs of X^T DMA prefetch lookahead


def body(nc, tc, outs, ins, cfg: CFG):
    """Emit the per-core program. ins/outs are dicts of DRAM APs."""
    X_d = ins["xw"]         # [NB, 128, L, C_IN] bf16  pre-transposed lhsT
    XC_d = ins["xc"]        # [L//2, 32, 1024] bf16  X^T pair-blocks (vh,l,v)
    G_d = ins["gt"]         # [128, GDEP, NB, 512] bf16  w-major G_k^T blocks
    W_d = ins["wt"]         # [128, C_OUT] bf16  W^T
    b_d = ins["bias2"]      # [128, 1]     f32   bias duplicated for (vh, o)
    out_d = outs["out"]     # [n_chunks, 128, Lc, 256] bf16 chunk-major

    Lc, Lq = cfg.Lc, cfg.Lq
    n_chunks = cfg.L // Lc
    n_xq = cfg.L // Lq

    XC_AHEAD = cfg.xc_ahead

    with (
        tc.tile_pool(name="const", bufs=1) as cpool,
        tc.tile_pool(name="h0sb", bufs=XC_AHEAD + 2) as h0sb_pool,
        tc.tile_pool(name="outsb", bufs=2) as out_pool,
        tc.tile_pool(name="h0ps", bufs=3, space="PSUM") as h0ps_pool,
        tc.tile_pool(name="cvps", bufs=3, space="PSUM") as cvps_pool,
        tc.tile_pool(name="wmps", bufs=1, space="PSUM") as wm_pool,
    ):
        # ---- DMA order is load-bearing: g00 feeds the warmup matmuls ----
        g00 = cpool.tile([128, N], BF16, name="g00")
        nc.sync.dma_start(g00[:], G_d[:, 0, 0, :])

        w_t = cpool.tile([128, C_OUT], BF16, name="w_t")
        nc.sync.dma_start(w_t[:], W_d[:])
        b_t = cpool.tile([128, 1], F32, name="b_t")
        nc.sync.dma_start(b_t[:], b_d[:])
        # G block tiles in 3 w-major DMAs (128 x 4 KB descriptors each)
        g_k = []
        for k in range(GDEP):
            t = cpool.tile([128, NB * N], BF16, name=f"g_k{k}")
            nc.sync.dma_start(
                t.rearrange("w (b v) -> w b v", b=NB), G_d[:, k]
            )
            g_k.append(t)
        g_t = [
            [g_k[k][:, wb * N:(wb + 1) * N] for wb in range(NB)]
            for k in range(GDEP)
        ]

        # HAM warmup on the PE while the bulk DMAs land; a dummy Scalar
        # activation pulls the ACT table load (~2.7 us) into the fill phase.
        sc_warm = cpool.tile([128, 1], F32, name="sc_warm")
        nc.scalar.add(sc_warm[:], b_t[:], 0.0)
        wm = wm_pool.tile([128, N], F32, name="wm")
        for _ in range(cfg.n_warm):
            nc.tensor.matmul(
                wm[:], lhsT=g00[:, 0:128], rhs=g00[:],
                start=True, stop=True,
            )

        # X slices needed first, then the remaining G tiles, then the rest.
        xw = [[None] * n_xq for _ in range(NB)]

        def load_xq(lq):
            for wb in range(NB):
                t = cpool.tile([128, Lq * C_IN], BF16, name=f"xw_{wb}_{lq}")
                nc.sync.dma_start(
                    t.rearrange("w (l c) -> w l c", c=C_IN),
                    X_d[wb, :, lq * Lq:(lq + 1) * Lq, :],
                )
                xw[wb][lq] = t

        # h0 staging per pair: one [128, (vh 2, l 2, v 256)] tile. Channel
        # order is (G1, G2, G3, X) -- W^T rows rolled on the host -- so the
        # 96-partition PSUM evac starts at partition 0 (quadrant alignment
        # rule); X^T pair-blocks are DMA'd into partitions 96:128 with
        # XC_AHEAD pairs of lookahead.
        h0S = {}

        def alloc_pair(p):
            if p >= cfg.L // 2:
                return
            s = h0sb_pool.tile([128, 1024], BF16, name="h0s2")
            nc.sync.dma_start(s[96:128, :], XC_d[p])
            h0S[p] = s

        def emit_conv(p, lp):
            """Channel-mix conv for the l-pair (lp, lp+1) + bias.

            Four m=32 quarter-matmuls on distinct 32-column strips -- the
            only packing the PE runs concurrently (m=64 pairs serialize).
            Quarter q covers (vh, o-half) with output partitions 32q:32q+32.
            """
            s = h0S.pop(p)
            cvp = cvps_pool.tile([128, 512], F32, name="cvp")
            for q in range(4):
                vh, oh = q // 2, q % 2
                nc.tensor.matmul(
                    cvp[32 * q:32 * (q + 1), :],
                    lhsT=w_t[:, 32 * oh:32 * (oh + 1)],
                    rhs=s[:, 512 * vh:512 * (vh + 1)],
                    start=True, stop=True, tile_position=(0, 32 * q),
                    skip_group_check=True,
                )
            dst = out_sb[:, (lp % Lc) * 256:(lp % Lc + 2) * 256]
            if p % 2 == 0:
                nc.scalar.add(dst, cvp[:], b_t[:, 0:1])
            else:
                nc.vector.tensor_scalar_add(
                    out=dst, in0=cvp[:], scalar1=b_t[:, 0:1]
                )

        # Upfront: a few latency-critical X^T prefetches, then X for the
        # first slice; the rest drips in one 512 KB piece per 16 l so the
        # DMA queues never develop a backlog that delays the prefetches.
        for p in range(6):
            alloc_pair(p)
        next_alloc = 6
        load_xq(0)
        drip = [(wb, lq) for lq in range(1, n_xq) for wb in range(NB)]

        out_sb = out_pool.tile([128, 256 * Lc], BF16, name="out_sb")
        for l in range(cfg.L):
            p = l // 2
            if l % 16 == 0 and l // 16 < len(drip):
                wb_d, lq_d = drip[l // 16]
                t = cpool.tile(
                    [128, Lq * C_IN], BF16, name=f"xw_{wb_d}_{lq_d}"
                )
                nc.sync.dma_start(
                    t.rearrange("w (l c) -> w l c", c=C_IN),
                    X_d[wb_d, :, lq_d * Lq:(lq_d + 1) * Lq, :],
                )
                xw[wb_d][lq_d] = t
            h0p = h0ps_pool.tile([128, N], F32, name="h0p")
            for wb in range(NB):
                st = wb == 0
                sp = wb == NB - 1
                xl = xw[wb][l // Lq][:, (l % Lq) * C_IN:(l % Lq + 1) * C_IN]
                for k in range(GDEP):
                    nc.tensor.matmul(
                        h0p[32 * k:32 * (k + 1), :], lhsT=xl,
                        rhs=g_t[k][wb][:],
                        start=st, stop=sp, tile_position=(0, 32 * k),
                        skip_group_check=True,
                    )
            # evac hop channels into the (vh 2, l 2, v 256) slots
            dst = h0S[p].rearrange("p (vh l v) -> p vh l v", vh=2, l=2)[
                0:96, :, l % 2, :
            ]
            if l % 2 == 0:
                nc.vector.tensor_copy(out=dst, in_=h0p[0:96, :])
            else:
                nc.scalar.copy(dst, h0p[0:96, :])

            if l % 2 == 1:
                for _ in range(2):  # catch up to XC_AHEAD pairs of prefetch
                    if next_alloc <= p + XC_AHEAD and next_alloc < cfg.L // 2:
                        alloc_pair(next_alloc)
                        next_alloc += 1
                if p > 0:
                    emit_conv(p - 1, l - 3)
                    if (l - 3) % Lc == Lc - 2:  # chunk complete -> store it
                        ch = (l - 3) // Lc
                        nc.sync.dma_start(
                            out_d[ch],
                            out_sb.rearrange("p (l v) -> p l v", v=256),
                        )
                        if ch + 1 < n_chunks:
                            out_sb = out_pool.tile(
                                [128, 256 * Lc], BF16, name="out_sb"
                            )
        emit_conv(cfg.L // 2 - 1, cfg.L - 2)
        nc.sync.dma_start(
            out_d[n_chunks - 1],
            out_sb.rearrange("p (l v) -> p l v", v=256),
        )


def build_nc(cfg: CFG):
    nc = bacc.Bacc("TRN2", target_bir_lowering=False, debug=False)
    n_chunks = cfg.L // cfg.Lc
    ins = {
        "xw": nc.dram_tensor("xw", [NB, 128, cfg.L, C_IN], BF16,
                             kind="ExternalInput").ap(),
        "xc": nc.dram_tensor("xc", [cfg.L // 2, 32, 1024], BF16,
                             kind="ExternalInput").ap(),
        "gt": nc.dram_tensor("gt", [128, GDEP, NB, 512], BF16,
                             kind="ExternalInput").ap(),
        "wt": nc.dram_tensor("wt", [128, C_OUT], BF16,
                             kind="ExternalInput").ap(),
        "bias2": nc.dram_tensor("bias2", [128, 1], F32,
                                kind="ExternalInput").ap(),
    }
    outs = {
        "out": nc.dram_tensor("out", [n_chunks, 128, cfg.Lc, 256], BF16,
                              kind="ExternalOutput").ap(),
    }
    with tile.TileContext(nc) as tc:
        body(nc, tc, outs, ins, cfg)
    nc.compile()
    return nc


def make_host_inputs(X, A, W, b):
    """Precompute all device operands on the host."""
    import ml_dtypes
    bf16 = ml_dtypes.bfloat16

    A = np.asarray(A, np.float64)
    n = A.shape[0]
    An = A + np.eye(n)
    An = An / An.sum(axis=1, keepdims=True)
    As = (1.0 - BETA) * An
    eye = np.eye(n)
    G = []
    gk = eye
    for _ in range(GDEP):
        gk = As @ gk + BETA * eye
        G.append(gk)
    GT = np.stack([g.T for g in G]).astype(bf16)  # [GDEP, N, N]
    # w-major G blocks [w 128, k GDEP, wb NB, v 512] for one big load
    GT = np.ascontiguousarray(
        GT.reshape(GDEP, NB, 128, N).transpose(2, 0, 1, 3)
    )
    # W^T rows rolled so the channel order is (G1, G2, G3, X), matching the
    # device-side concat layout (hops in PSUM partitions 0:96, X DMA'd into
    # 96:128).
    WT = np.roll(np.asarray(W, np.float64).T, -C_IN, axis=0)
    WT = np.ascontiguousarray(WT.astype(bf16))
    b = np.asarray(b, np.float32)
    b2 = np.ascontiguousarray(np.concatenate([b, b]).reshape(128, 1))

    # X [B, C_IN, N, L] f32 -> per core [NB, 128, L, C_IN] bf16 (lhsT layout)
    X = np.asarray(X)
    XW = np.ascontiguousarray(X.transpose(0, 2, 3, 1)).astype(bf16)
    XW = XW.reshape(B, NB, 128, L, C_IN)
    # X^T pair-blocks [L//2, c 32, (vh 2, l 2, v 256)] bf16 for direct DMA
    # into the h0 staging tiles' partitions 96:128.
    XC = X.reshape(B, C_IN, 2, 256, L // 2, 2).transpose(0, 4, 1, 2, 5, 3)
    XC = np.ascontiguousarray(XC).astype(bf16).reshape(B, L // 2, 32, 1024)
    return XW, XC, GT, WT, b2


_NC_CACHE = {}


def run_on_hw(X, A, W, b, cfg=None, trace=False, **spmd_kwargs):
    XW, XC, GT, WT, b2 = make_host_inputs(X, A, W, b)
    if cfg is None:
        cfg = CFG()
    key = (cfg.L, cfg.Lc, cfg.Lq, cfg.n_warm)
    if key not in _NC_CACHE:
        _NC_CACHE[key] = build_nc(cfg)
    nc = _NC_CACHE[key]
    in_maps = [
        {"xw": XW[i], "xc": XC[i], "gt": GT, "wt": WT, "bias2": b2}
        for i in range(B)
    ]
    res = bass_utils.run_bass_kernel_spmd(
        nc, in_maps, core_ids=list(range(B)), trace=trace, **spmd_kwargs
    )
    # out_dev [n_chunks, 128=(vh,o), Lc, 256=v] bf16
    #   -> out [C_OUT, N, L] f32  via (o, vh, v, ch, l)
    n_chunks = cfg.L // cfg.Lc
    outs = []
    for i in range(B):
        o = np.asarray(res.results[i]["out"])
        o = o.reshape(n_chunks, 2, C_OUT, cfg.Lc, 256)
        o = o.transpose(2, 1, 4, 0, 3).reshape(C_OUT, N, cfg.L)
        outs.append(o.astype(np.float32))
    return np.stack(outs), res


def kernel(X, A, W, b):
    return run_on_hw(X, A, W, b)[0]


if __name__ == "__main__":
    rng = np.random.default_rng(0)
    X = rng.standard_normal((B, C_IN, N, L), dtype=np.float32)
    A = rng.random((N, N), dtype=np.float32)
    W = rng.standard_normal((C_OUT, (GDEP + 1) * C_IN), dtype=np.float32) * 0.1
    b = rng.random(C_OUT, dtype=np.float32)
    out = kernel(X, A, W, b)
    print("out", out.shape, out.dtype, float(np.abs(out).mean()))
